# revision 15
# baseline (speedup 1.0000x reference)
"""Pairwise cosine similarity  O = (Z/|Z_rows|) @ (Y/|Y_rows|).T  on 8 TRN2 cores.

v19: zero on-device transposes. The host uploads BOTH operands in
PE-ready (feature-major) layout, so the device program is a pure
load -> matmul -> scale pipeline. Measured 261106 ns.
"""

import contextlib
import os
import sys
import numpy as np

_TRN_REPO = "/opt/trn_rl_repo"
if _TRN_REPO not in sys.path:
    sys.path.insert(0, _TRN_REPO)

import concourse.bacc as bacc
import concourse.mybir as mybir
import concourse.tile as tile
from concourse.bass_utils import run_bass_kernel_spmd

P = 128
N_CORES = 8
F32 = mybir.dt.float32
BF16 = mybir.dt.bfloat16

FEAT = 4096
BZ = 1024           # z rows per core (4 z-shards)
BY = 2048           # y rows per core (2 y-shards)
K_TILES = FEAT // P          # 32
S_SLICES = 8                 # yt column slices of 256 y
C_COLS = 2                   # 128-y stationary columns per slice
H_SLICES = BZ // 512         # 2 moving slices of 512 z
UNROLL = 2


def build(bench_iters=None):
    """Build + bacc-compile the SPMD program (same program on every core)."""
    nc = bacc.Bacc("TRN2", target_bir_lowering=False, debug=False,
                   num_devices=N_CORES)
    if bench_iters is None:
        zt = nc.dram_tensor("zt", [P, K_TILES, BZ], BF16,
                            kind="ExternalInput").ap()
        yt = nc.dram_tensor("yt", [S_SLICES * P, K_TILES, 256], BF16,
                            kind="ExternalInput").ap()
        ynat = nc.dram_tensor("ynat", [BY, FEAT], BF16,
                              kind="ExternalInput").ap()
        o = nc.dram_tensor("o", [BY, BZ], BF16, kind="ExternalOutput").ap()
    else:
        zt = nc.dram_tensor("zti", [P, K_TILES, BZ], BF16).ap()
        yt = nc.dram_tensor("yti", [S_SLICES * P, K_TILES, 256], BF16).ap()
        ynat = nc.dram_tensor("ynati", [BY, FEAT], BF16).ap()
        o = nc.dram_tensor("oi", [BY, BZ], BF16).ap()
        dummy_in = nc.dram_tensor("dummy_in", [1, 64], F32,
                                  kind="ExternalInput").ap()
        dummy_out = nc.dram_tensor("dummy_out", [1, 64], F32,
                                   kind="ExternalOutput").ap()

    with tile.TileContext(nc) as tc:
        with tc.tile_pool(name="const", bufs=1) as const_pool, \
             tc.tile_pool(name="zt", bufs=2) as zt_pool, \
             tc.tile_pool(name="yt", bufs=2) as yt_pool, \
             tc.tile_pool(name="ynat", bufs=2) as ynat_pool, \
             tc.tile_pool(name="z2", bufs=2) as z2_pool, \
             tc.tile_pool(name="nzb", bufs=2) as nzb_pool, \
             tc.tile_pool(name="ny", bufs=2) as ny_pool, \
             tc.tile_pool(name="small", bufs=2) as small_pool, \
             tc.tile_pool(name="outs", bufs=2) as out_pool, \
             tc.tile_pool(name="pacc", bufs=3, space="PSUM") as pacc_pool, \
             tc.tile_pool(name="pnz", bufs=1, space="PSUM") as pnz_pool:

            ones = const_pool.tile([P, P], BF16, name="ones")
            nc.vector.memset(ones[:], 1.0)

            def body():
                # ---- loads: zt (8MB) then the 8 yt slices on sync ----
                ztile = zt_pool.tile([P, K_TILES, BZ], BF16, tag="zt")
                nc.sync.dma_start(out=ztile[:], in_=zt[:])
                ytiles = []
                for s in range(S_SLICES):
                    yt_t = yt_pool.tile([P, K_TILES, 256], BF16, tag="yt")
                    nc.sync.dma_start(out=yt_t[:],
                                      in_=yt[s * P:(s + 1) * P, :, :])
                    ytiles.append(yt_t)

                # ---- Y natural-layout loads (gpsimd queue, early burst;
                # squares are interleaved into the main loop below) ----
                ny = ny_pool.tile([P, BY // P], F32, tag="ny")
                yns = []
                for yi in range(BY // P):
                    yn = ynat_pool.tile([P, FEAT], BF16, tag="ynat")
                    nc.gpsimd.dma_start(
                        out=yn[:], in_=ynat[yi * P:(yi + 1) * P, :])
                    yns.append(yn)

                def y_rnorm(yi):
                    ss = small_pool.tile([P, 1], F32, tag="ss")
                    # in-place square; we only need the row sums
                    nc.scalar.activation(
                        yns[yi][:], yns[yi][:],
                        mybir.ActivationFunctionType.Square,
                        accum_out=ss[:])
                    std = small_pool.tile([P, 1], F32, tag="std")
                    nc.scalar.sqrt(std[:], ss[:])
                    nc.vector.reciprocal(ny[:, yi:yi + 1], std[:])

                # ---- Z norms: ones-stationary matmuls over z^2 ----
                # nzacc[h][p, z] = sum_feat z^2  (identical on every p)
                nzacc = [pnz_pool.tile([P, 512], F32, tag=f"nz{h}",
                                       name=f"nz{h}")
                         for h in range(H_SLICES)]
                for k in range(K_TILES):
                    z2 = z2_pool.tile([P, BZ], BF16, tag="z2")
                    nc.vector.tensor_mul(z2[:], ztile[:, k, :],
                                         ztile[:, k, :])
                    for h in range(H_SLICES):
                        nc.tensor.matmul(
                            nzacc[h][:], ones[:],
                            z2[:, h * 512:(h + 1) * 512],
                            start=(k == 0), stop=(k == K_TILES - 1))
                nzb = nzb_pool.tile([P, BZ], F32, tag="nzb")
                for h in range(H_SLICES):
                    nc.scalar.sqrt(nzb[:, h * 512:(h + 1) * 512],
                                   nzacc[h][:])
                nc.vector.reciprocal(nzb[:], nzb[:])

                # first two Y norms here: after the z2 squares in DVE's
                # stream (so PE's norm matmuls start immediately), before
                # slice 0's evictions need ny[0]/ny[1]
                y_rnorm(0)
                y_rnorm(1)

                # ---- main matmuls + eviction ----
                for s in range(S_SLICES):
                    accs = {}
                    for c in range(C_COLS):
                        for h in range(H_SLICES):
                            accs[c, h] = pacc_pool.tile(
                                [P, 512], F32, tag=f"acc{h}",
                                name=f"acc{h}")
                    for k in range(K_TILES):
                        for c in range(C_COLS):
                            lhsT = ytiles[s][:, k, c * P:(c + 1) * P]
                            for h in range(H_SLICES):
                                nc.tensor.matmul(
                                    accs[c, h][:], lhsT,
                                    ztile[:, k, h * 512:(h + 1) * 512],
                                    start=(k == 0),
                                    stop=(k == K_TILES - 1))
                    for c in range(C_COLS):
                        yi = s * C_COLS + c
                        ob = out_pool.tile([P, BZ], BF16, tag="ob")
                        for h in range(H_SLICES):
                            nc.scalar.activation(
                                ob[:, h * 512:(h + 1) * 512],
                                accs[c, h][:],
                                mybir.ActivationFunctionType.Copy,
                                scale=ny[:, yi:yi + 1])
                            nc.vector.tensor_mul(
                                ob[:, h * 512:(h + 1) * 512],
                                ob[:, h * 512:(h + 1) * 512],
                                nzb[:, h * 512:(h + 1) * 512])
                        nc.scalar.dma_start(
                            out=o[yi * P:(yi + 1) * P, :], in_=ob[:])
                    # next slice's Y-norm squares follow this slice's
                    # evictions in ACT's in-order stream; their loads
                    # were issued in the early burst so there are no
                    # fresh-DMA waits here
                    if s < S_SLICES - 1:
                        y_rnorm((s + 1) * C_COLS)
                        y_rnorm((s + 1) * C_COLS + 1)

            if bench_iters is None:
                body()
            else:
                assert bench_iters % UNROLL == 0
                with tc.For_i(0, bench_iters // UNROLL, 1):
                    for _ in range(UNROLL):
                        body()

            if bench_iters is not None:
                db = small_pool.tile([1, 64], F32, tag="db", name="db")
                nc.gpsimd.dma_start(out=db[:], in_=dummy_in[:])
                nc.vector.tensor_copy(db[:], db[:])
                nc.gpsimd.dma_start(out=dummy_out[:], in_=db[:])

    nc.compile()
    return nc


_CACHE = {}


def _get_compiled():
    if "nc" not in _CACHE:
        _CACHE["nc"] = build()
    return _CACHE["nc"]


def kernel(Z, Y):
    from ml_dtypes import bfloat16
    Z = np.asarray(Z, dtype=np.float32).astype(bfloat16)
    Y = np.asarray(Y, dtype=np.float32).astype(bfloat16)
    bz_full, by_full = Z.shape[0], Y.shape[0]
    zs, ys = bz_full // 4, by_full // 2          # 4x2 grid shards
    nc = _get_compiled()
    in_maps = []
    for i in range(N_CORES):
        zi, yi = divmod(i, 2)
        zsh = Z[zi * zs:(zi + 1) * zs]           # [1024, 4096]
        ysh = Y[yi * ys:(yi + 1) * ys]           # [2048, 4096]
        # zt[p, k, z] = zsh[z, k*128+p]
        ztd = np.ascontiguousarray(
            zsh.T.reshape(K_TILES, P, zs).transpose(1, 0, 2))
        # yt[s*128+p, k, y] = ysh[s*256+y, k*128+p]
        ytd = np.ascontiguousarray(
            ysh.T.reshape(K_TILES, P, S_SLICES, 256)
               .transpose(2, 1, 0, 3).reshape(S_SLICES * P, K_TILES, 256))
        in_maps.append({"zt": ztd, "yt": ytd,
                        "ynat": np.ascontiguousarray(ysh)})
    res = run_bass_kernel_spmd(nc, in_maps, list(range(N_CORES)))
    # core (zi, yi) returns O^T block [ys, zs] in bf16; assemble + upcast
    out_t = np.empty((by_full, bz_full), dtype=np.float32)
    for i in range(N_CORES):
        zi, yi = divmod(i, 2)
        out_t[yi * ys:(yi + 1) * ys, zi * zs:(zi + 1) * zs] = \
            res.results[i]["o"].astype(np.float32)
    return np.ascontiguousarray(out_t.T)


# revision 16
# speedup vs baseline: 1.0952x; 1.0952x over previous
"""Pairwise cosine similarity  O = (Z/|Z_rows|) @ (Y/|Y_rows|).T  on 8 TRN2 cores.

v19: zero on-device transposes. The host uploads BOTH operands in
PE-ready (feature-major) layout, so the device program is a pure
load -> matmul -> scale pipeline. Measured 261106 ns.
"""

import contextlib
import os
import sys
import numpy as np

_TRN_REPO = "/opt/trn_rl_repo"
if _TRN_REPO not in sys.path:
    sys.path.insert(0, _TRN_REPO)

import concourse.bacc as bacc
import concourse.mybir as mybir
import concourse.tile as tile
from concourse.bass_utils import run_bass_kernel_spmd

P = 128
N_CORES = 8
F32 = mybir.dt.float32
BF16 = mybir.dt.bfloat16

FEAT = 4096
BZ = 1024           # z rows per core (4 z-shards)
BY = 2048           # y rows per core (2 y-shards)
K_TILES = FEAT // P          # 32
S_SLICES = 8                 # yt column slices of 256 y
C_COLS = 2                   # 128-y stationary columns per slice
H_SLICES = BZ // 512         # 2 moving slices of 512 z
UNROLL = 2


def build(bench_iters=None):
    """Build + bacc-compile the SPMD program (same program on every core)."""
    nc = bacc.Bacc("TRN2", target_bir_lowering=False, debug=False,
                   num_devices=N_CORES)
    if bench_iters is None:
        zt = nc.dram_tensor("zt", [P, K_TILES, BZ], BF16,
                            kind="ExternalInput").ap()
        yt = nc.dram_tensor("yt", [S_SLICES * P, K_TILES, 256], BF16,
                            kind="ExternalInput").ap()
        ynat = nc.dram_tensor("ynat", [BY, FEAT], BF16,
                              kind="ExternalInput").ap()
        o = nc.dram_tensor("o", [BY, BZ], BF16, kind="ExternalOutput").ap()
    else:
        zt = nc.dram_tensor("zti", [P, K_TILES, BZ], BF16).ap()
        yt = nc.dram_tensor("yti", [S_SLICES * P, K_TILES, 256], BF16).ap()
        ynat = nc.dram_tensor("ynati", [BY, FEAT], BF16).ap()
        o = nc.dram_tensor("oi", [BY, BZ], BF16).ap()
        dummy_in = nc.dram_tensor("dummy_in", [1, 64], F32,
                                  kind="ExternalInput").ap()
        dummy_out = nc.dram_tensor("dummy_out", [1, 64], F32,
                                   kind="ExternalOutput").ap()

    with tile.TileContext(nc) as tc:
        with tc.tile_pool(name="const", bufs=1) as const_pool, \
             tc.tile_pool(name="zt", bufs=2) as zt_pool, \
             tc.tile_pool(name="yt", bufs=2) as yt_pool, \
             tc.tile_pool(name="ynat", bufs=2) as ynat_pool, \
             tc.tile_pool(name="z2", bufs=2) as z2_pool, \
             tc.tile_pool(name="nzb", bufs=2) as nzb_pool, \
             tc.tile_pool(name="ny", bufs=2) as ny_pool, \
             tc.tile_pool(name="small", bufs=2) as small_pool, \
             tc.tile_pool(name="outs", bufs=2) as out_pool, \
             tc.tile_pool(name="pacc", bufs=3, space="PSUM") as pacc_pool, \
             tc.tile_pool(name="pnz", bufs=1, space="PSUM") as pnz_pool:

            ones = const_pool.tile([P, P], BF16, name="ones")
            nc.vector.memset(ones[:], 1.0)

            def body():
                # ---- loads: zt (8MB) then the 8 yt slices on sync ----
                ztile = zt_pool.tile([P, K_TILES, BZ], BF16, tag="zt")
                nc.sync.dma_start(out=ztile[:], in_=zt[:])
                ytiles = []
                for s in range(S_SLICES):
                    yt_t = yt_pool.tile([P, K_TILES, 256], BF16, tag="yt")
                    nc.sync.dma_start(out=yt_t[:],
                                      in_=yt[s * P:(s + 1) * P, :, :])
                    ytiles.append(yt_t)

                # ---- Y row norms from natural layout (gpsimd queue) ----
                ny = ny_pool.tile([P, BY // P], F32, tag="ny")
                for yi in range(BY // P):
                    yn = ynat_pool.tile([P, FEAT], BF16, tag="ynat")
                    nc.gpsimd.dma_start(
                        out=yn[:], in_=ynat[yi * P:(yi + 1) * P, :])
                    ss = small_pool.tile([P, 1], F32, tag="ss")
                    # in-place square; we only need the row sums
                    nc.scalar.activation(
                        yn[:], yn[:],
                        mybir.ActivationFunctionType.Square,
                        accum_out=ss[:])
                    std = small_pool.tile([P, 1], F32, tag="std")
                    nc.scalar.sqrt(std[:], ss[:])
                    nc.vector.reciprocal(ny[:, yi:yi + 1], std[:])

                # ---- Z norms: ones-stationary matmuls over z^2 ----
                # nzacc[h][p, z] = sum_feat z^2  (identical on every p)
                nzacc = [pnz_pool.tile([P, 512], F32, tag=f"nz{h}",
                                       name=f"nz{h}")
                         for h in range(H_SLICES)]
                for k in range(K_TILES):
                    z2 = z2_pool.tile([P, BZ], BF16, tag="z2")
                    nc.vector.tensor_mul(z2[:], ztile[:, k, :],
                                         ztile[:, k, :])
                    for h in range(H_SLICES):
                        nc.tensor.matmul(
                            nzacc[h][:], ones[:],
                            z2[:, h * 512:(h + 1) * 512],
                            start=(k == 0), stop=(k == K_TILES - 1))
                nzb = nzb_pool.tile([P, BZ], F32, tag="nzb")
                for h in range(H_SLICES):
                    nc.scalar.sqrt(nzb[:, h * 512:(h + 1) * 512],
                                   nzacc[h][:])
                nc.vector.reciprocal(nzb[:], nzb[:])

                # ---- main matmuls + eviction ----
                for s in range(S_SLICES):
                    accs = {}
                    for c in range(C_COLS):
                        for h in range(H_SLICES):
                            accs[c, h] = pacc_pool.tile(
                                [P, 512], F32, tag=f"acc{h}",
                                name=f"acc{h}")
                    for k in range(K_TILES):
                        for c in range(C_COLS):
                            lhsT = ytiles[s][:, k, c * P:(c + 1) * P]
                            for h in range(H_SLICES):
                                nc.tensor.matmul(
                                    accs[c, h][:], lhsT,
                                    ztile[:, k, h * 512:(h + 1) * 512],
                                    start=(k == 0),
                                    stop=(k == K_TILES - 1))
                    for c in range(C_COLS):
                        yi = s * C_COLS + c
                        ob = out_pool.tile([P, BZ], BF16, tag="ob")
                        for h in range(H_SLICES):
                            nc.scalar.activation(
                                ob[:, h * 512:(h + 1) * 512],
                                accs[c, h][:],
                                mybir.ActivationFunctionType.Copy,
                                scale=ny[:, yi:yi + 1])
                            nc.vector.tensor_mul(
                                ob[:, h * 512:(h + 1) * 512],
                                ob[:, h * 512:(h + 1) * 512],
                                nzb[:, h * 512:(h + 1) * 512])
                        nc.scalar.dma_start(
                            out=o[yi * P:(yi + 1) * P, :], in_=ob[:])

            if bench_iters is None:
                body()
            else:
                assert bench_iters % UNROLL == 0
                with tc.For_i(0, bench_iters // UNROLL, 1):
                    for _ in range(UNROLL):
                        body()

            if bench_iters is not None:
                db = small_pool.tile([1, 64], F32, tag="db", name="db")
                nc.gpsimd.dma_start(out=db[:], in_=dummy_in[:])
                nc.vector.tensor_copy(db[:], db[:])
                nc.gpsimd.dma_start(out=dummy_out[:], in_=db[:])

    nc.compile()
    return nc


_CACHE = {}


def _get_compiled():
    if "nc" not in _CACHE:
        _CACHE["nc"] = build()
    return _CACHE["nc"]


def kernel(Z, Y):
    from ml_dtypes import bfloat16
    Z = np.asarray(Z, dtype=np.float32).astype(bfloat16)
    Y = np.asarray(Y, dtype=np.float32).astype(bfloat16)
    bz_full, by_full = Z.shape[0], Y.shape[0]
    zs, ys = bz_full // 4, by_full // 2          # 4x2 grid shards
    nc = _get_compiled()
    in_maps = []
    for i in range(N_CORES):
        zi, yi = divmod(i, 2)
        zsh = Z[zi * zs:(zi + 1) * zs]           # [1024, 4096]
        ysh = Y[yi * ys:(yi + 1) * ys]           # [2048, 4096]
        # zt[p, k, z] = zsh[z, k*128+p]
        ztd = np.ascontiguousarray(
            zsh.T.reshape(K_TILES, P, zs).transpose(1, 0, 2))
        # yt[s*128+p, k, y] = ysh[s*256+y, k*128+p]
        ytd = np.ascontiguousarray(
            ysh.T.reshape(K_TILES, P, S_SLICES, 256)
               .transpose(2, 1, 0, 3).reshape(S_SLICES * P, K_TILES, 256))
        in_maps.append({"zt": ztd, "yt": ytd,
                        "ynat": np.ascontiguousarray(ysh)})
    res = run_bass_kernel_spmd(nc, in_maps, list(range(N_CORES)))
    # core (zi, yi) returns O^T block [ys, zs] in bf16; assemble + upcast
    out_t = np.empty((by_full, bz_full), dtype=np.float32)
    for i in range(N_CORES):
        zi, yi = divmod(i, 2)
        out_t[yi * ys:(yi + 1) * ys, zi * zs:(zi + 1) * zs] = \
            res.results[i]["o"].astype(np.float32)
    return np.ascontiguousarray(out_t.T)


# revision 18
# speedup vs baseline: 1.3472x; 1.2301x over previous
"""Pairwise cosine similarity  O = (Z/|Z_rows|) @ (Y/|Y_rows|).T  on 8 TRN2 cores.

v21: zero on-device transposes of the operands; Z-norm broadcast via a
tiny transpose-dance instead of 64 ones-matmuls (saves ~13.6us of PE).

  - 4x2 output grid: core (zi, yi) holds a 1024-row Z shard and a
    2048-row Y shard and computes the O^T block [2048, 1024].
  - The host uploads BOTH operands in PE-ready (feature-major) layout:
    zt DRAM [128, 32, 1024] bf16 == Z_shard^T tiled (partition p holds
    feat rows {k*128+p}); ONE contiguous 8MB DMA, SBUF-resident.
    yt DRAM [1024, 32, 256] bf16 == Y_shard^T tiled into 8 column
    slices of 256 y; streamed, 2MB contiguous DMAs on the sync ring.
  - main MMs: stationary = yt[:, k, c*128:(c+1)*128] (128 y columns),
    moving = zt[:, k, h*512:(h+1)*512]; acc[c,h] = [128 y, 512 z] PSUM,
    accumulated over 32 k-tiles. 1024 MMs of N=512 -> ~218us roofline.
  - 1/|z| and 1/|y| both come from natural-layout loads (gpsimd ring)
    + scalar-engine Square with accum_out, as one early burst per body.
    1/|y| [128,1] per y-tile is the activation scale at eviction.
    1/|z| needs free-dim-broadcast form: 8 reciprocal columns packed
    [128, 8], PE-transposed to [8, 128], flattened to [1, 1024] by a
    tiny SBUF->SBUF DMA, broadcast to [128, 1024] with two K=1
    ones-stationary rank-1 matmuls.
  - eviction: PSUM -> (ACT copy * ny_inv) -> (DVE mul nz_bcast) -> DMA
    on the scalar ring. (A fused scalar_tensor_tensor eviction was
    benched and lost: DVE reads PSUM without perf modes and holds the
    acc bank longer.)
  - bench mode unrolls U=2 bodies per For_i iteration so the next
    body's loads overlap this body's tail matmuls (the For_i back edge
    is a full barrier; UNROLL=4 regressed badly on HW).
  - measured (interleaved same-window A/B vs alternatives): floor slope
    277us vs 307us for the no-dance variant; best clean-window reading
    of this structure's family was 261-329us (device throughput drifts
    ~1.5x across hours, so absolute numbers vary by window).
"""

import contextlib
import os
import sys
import numpy as np

_TRN_REPO = "/opt/trn_rl_repo"
if _TRN_REPO not in sys.path:
    sys.path.insert(0, _TRN_REPO)

import concourse.bacc as bacc
import concourse.mybir as mybir
import concourse.tile as tile
from concourse.bass_utils import run_bass_kernel_spmd
from concourse.masks import make_identity

P = 128
N_CORES = 8
F32 = mybir.dt.float32
BF16 = mybir.dt.bfloat16

FEAT = 4096
BZ = 1024           # z rows per core (4 z-shards)
BY = 2048           # y rows per core (2 y-shards)
K_TILES = FEAT // P          # 32
S_SLICES = 8                 # yt column slices of 256 y
C_COLS = 2                   # 128-y stationary columns per slice
H_SLICES = BZ // 512         # 2 moving slices of 512 z
Z_TILES = BZ // P            # 8
UNROLL = 2


def build(bench_iters=None):
    """Build + bacc-compile the SPMD program (same program on every core)."""
    nc = bacc.Bacc("TRN2", target_bir_lowering=False, debug=False,
                   num_devices=N_CORES)
    if bench_iters is None:
        zt = nc.dram_tensor("zt", [P, K_TILES, BZ], BF16,
                            kind="ExternalInput").ap()
        yt = nc.dram_tensor("yt", [S_SLICES * P, K_TILES, 256], BF16,
                            kind="ExternalInput").ap()
        znat = nc.dram_tensor("znat", [BZ, FEAT], BF16,
                              kind="ExternalInput").ap()
        ynat = nc.dram_tensor("ynat", [BY, FEAT], BF16,
                              kind="ExternalInput").ap()
        o = nc.dram_tensor("o", [BY, BZ], BF16, kind="ExternalOutput").ap()
    else:
        zt = nc.dram_tensor("zti", [P, K_TILES, BZ], BF16).ap()
        yt = nc.dram_tensor("yti", [S_SLICES * P, K_TILES, 256], BF16).ap()
        znat = nc.dram_tensor("znati", [BZ, FEAT], BF16).ap()
        ynat = nc.dram_tensor("ynati", [BY, FEAT], BF16).ap()
        o = nc.dram_tensor("oi", [BY, BZ], BF16).ap()
        dummy_in = nc.dram_tensor("dummy_in", [1, 64], F32,
                                  kind="ExternalInput").ap()
        dummy_out = nc.dram_tensor("dummy_out", [1, 64], F32,
                                   kind="ExternalOutput").ap()

    with tile.TileContext(nc) as tc:
        with tc.tile_pool(name="const", bufs=1) as const_pool, \
             tc.tile_pool(name="zt", bufs=2) as zt_pool, \
             tc.tile_pool(name="yt", bufs=2) as yt_pool, \
             tc.tile_pool(name="nat", bufs=3) as nat_pool, \
             tc.tile_pool(name="nzb", bufs=2) as nzb_pool, \
             tc.tile_pool(name="ny", bufs=2) as ny_pool, \
             tc.tile_pool(name="pk", bufs=2) as pk_pool, \
             tc.tile_pool(name="small", bufs=2) as small_pool, \
             tc.tile_pool(name="outs", bufs=2) as out_pool, \
             tc.tile_pool(name="pacc", bufs=3, space="PSUM") as pacc_pool, \
             tc.tile_pool(name="pnz", bufs=1, space="PSUM") as pnz_pool:

            ones = const_pool.tile([P, P], BF16, name="ones")
            nc.vector.memset(ones[:], 1.0)
            identf = const_pool.tile([P, P], F32, name="identf")
            make_identity(nc, identf)

            def row_rnorm(nat_ap, rdst):
                """rdst[p, 0] = 1/|row p| for a [P, FEAT] bf16 tile.
                Squares in place (the tile is only needed for norms)."""
                ss = small_pool.tile([P, 1], F32, tag="ss")
                nc.scalar.activation(
                    nat_ap, nat_ap,
                    mybir.ActivationFunctionType.Square,
                    accum_out=ss[:])
                std = small_pool.tile([P, 1], F32, tag="std")
                nc.scalar.sqrt(std[:], ss[:])
                nc.vector.reciprocal(rdst, std[:])

            def body():
                # ---- loads: zt (8MB) then the 8 yt slices on sync ----
                ztile = zt_pool.tile([P, K_TILES, BZ], BF16, tag="zt")
                nc.sync.dma_start(out=ztile[:], in_=zt[:])
                ytiles = []
                for s in range(S_SLICES):
                    yt_t = yt_pool.tile([P, K_TILES, 256], BF16, tag="yt")
                    nc.sync.dma_start(out=yt_t[:],
                                      in_=yt[s * P:(s + 1) * P, :, :])
                    ytiles.append(yt_t)

                # ---- Z row norms -> free-dim broadcast via dance ----
                pack = pk_pool.tile([P, Z_TILES], F32, tag="pack")
                for zi in range(Z_TILES):
                    zn = nat_pool.tile([P, FEAT], BF16, tag="nat")
                    nc.gpsimd.dma_start(
                        out=zn[:], in_=znat[zi * P:(zi + 1) * P, :])
                    row_rnorm(zn[:], pack[:, zi:zi + 1])
                # pack [128, 8] --PE transpose--> [8, 128] --flatten DMA-->
                # [1, 1024] --rank-1 ones matmuls--> nzb [128, 1024]
                trp = pnz_pool.tile([Z_TILES, P], F32, tag="trp", name="trp")
                nc.tensor.transpose(trp[:], pack[:], identf[:])
                tp = pk_pool.tile([Z_TILES, P], BF16, tag="tp")
                nc.vector.tensor_copy(tp[:], trp[:])
                flat = pk_pool.tile([1, BZ], BF16, tag="flat")
                nc.scalar.dma_start(out=flat[:], in_=tp[:])
                nzb = nzb_pool.tile([P, BZ], F32, tag="nzb")
                for h in range(H_SLICES):
                    nzp = pnz_pool.tile([P, 512], F32, tag="nzp",
                                        name="nzp")
                    nc.tensor.matmul(nzp[:], ones[0:1, :],
                                     flat[0:1, h * 512:(h + 1) * 512],
                                     start=True, stop=True)
                    nc.vector.tensor_copy(nzb[:, h * 512:(h + 1) * 512],
                                          nzp[:])

                # ---- Y row norms burst (v19-proven ordering) ----
                ny = ny_pool.tile([P, BY // P], F32, tag="ny")
                for yi in range(BY // P):
                    yn = nat_pool.tile([P, FEAT], BF16, tag="nat")
                    nc.gpsimd.dma_start(
                        out=yn[:], in_=ynat[yi * P:(yi + 1) * P, :])
                    row_rnorm(yn[:], ny[:, yi:yi + 1])

                # ---- main matmuls + eviction ----
                for s in range(S_SLICES):
                    accs = {}
                    for c in range(C_COLS):
                        for h in range(H_SLICES):
                            accs[c, h] = pacc_pool.tile(
                                [P, 512], F32, tag=f"acc{h}",
                                name=f"acc{h}")
                    for k in range(K_TILES):
                        for c in range(C_COLS):
                            lhsT = ytiles[s][:, k, c * P:(c + 1) * P]
                            for h in range(H_SLICES):
                                nc.tensor.matmul(
                                    accs[c, h][:], lhsT,
                                    ztile[:, k, h * 512:(h + 1) * 512],
                                    start=(k == 0),
                                    stop=(k == K_TILES - 1))
                    for c in range(C_COLS):
                        yi = s * C_COLS + c
                        ob = out_pool.tile([P, BZ], BF16, tag="ob")
                        for h in range(H_SLICES):
                            nc.scalar.activation(
                                ob[:, h * 512:(h + 1) * 512],
                                accs[c, h][:],
                                mybir.ActivationFunctionType.Copy,
                                scale=ny[:, yi:yi + 1])
                            nc.vector.tensor_mul(
                                ob[:, h * 512:(h + 1) * 512],
                                ob[:, h * 512:(h + 1) * 512],
                                nzb[:, h * 512:(h + 1) * 512])
                        nc.scalar.dma_start(
                            out=o[yi * P:(yi + 1) * P, :], in_=ob[:])

            if bench_iters is None:
                body()
            else:
                assert bench_iters % UNROLL == 0
                with tc.For_i(0, bench_iters // UNROLL, 1):
                    for _ in range(UNROLL):
                        body()

            if bench_iters is not None:
                db = small_pool.tile([1, 64], F32, tag="db", name="db")
                nc.gpsimd.dma_start(out=db[:], in_=dummy_in[:])
                nc.vector.tensor_copy(db[:], db[:])
                nc.gpsimd.dma_start(out=dummy_out[:], in_=db[:])

    nc.compile()
    return nc


_CACHE = {}


def _get_compiled():
    if "nc" not in _CACHE:
        _CACHE["nc"] = build()
    return _CACHE["nc"]


def kernel(Z, Y):
    from ml_dtypes import bfloat16
    Z = np.asarray(Z, dtype=np.float32).astype(bfloat16)
    Y = np.asarray(Y, dtype=np.float32).astype(bfloat16)
    bz_full, by_full = Z.shape[0], Y.shape[0]
    zs, ys = bz_full // 4, by_full // 2          # 4x2 grid shards
    nc = _get_compiled()
    in_maps = []
    for i in range(N_CORES):
        zi, yi = divmod(i, 2)
        zsh = Z[zi * zs:(zi + 1) * zs]           # [1024, 4096]
        ysh = Y[yi * ys:(yi + 1) * ys]           # [2048, 4096]
        # zt[p, k, z] = zsh[z, k*128+p]
        ztd = np.ascontiguousarray(
            zsh.T.reshape(K_TILES, P, zs).transpose(1, 0, 2))
        # yt[s*128+p, k, y] = ysh[s*256+y, k*128+p]
        ytd = np.ascontiguousarray(
            ysh.T.reshape(K_TILES, P, S_SLICES, 256)
               .transpose(2, 1, 0, 3).reshape(S_SLICES * P, K_TILES, 256))
        in_maps.append({"zt": ztd, "yt": ytd,
                        "znat": np.ascontiguousarray(zsh),
                        "ynat": np.ascontiguousarray(ysh)})
    res = run_bass_kernel_spmd(nc, in_maps, list(range(N_CORES)))
    # core (zi, yi) returns O^T block [ys, zs] in bf16; assemble + upcast
    out_t = np.empty((by_full, bz_full), dtype=np.float32)
    for i in range(N_CORES):
        zi, yi = divmod(i, 2)
        out_t[yi * ys:(yi + 1) * ys, zi * zs:(zi + 1) * zs] = \
            res.results[i]["o"].astype(np.float32)
    return np.ascontiguousarray(out_t.T)
